# revision 16
# baseline (speedup 1.0000x reference)
"""Multi-head attention (lazy K/V projections) Trainium2 Bass kernel.

Problem: nn_MultiHeadAttention_54520314856024
  B=8, SQ=SK=1024, D=1024, E=128, H=32
  keys  = einsum('bsd,hde->hbse', states, Wk) + bk
  vals  = einsum('bsd,hde->hbse', states, Wv) + bv
  attn  = softmax(einsum('bqe,hbke->hbqk', query, keys) / sqrt(E))
  ctx   = einsum('hbqk,hbke->hbqe', attn, vals) -> concat heads -> @ Wc + bc
  This is out = sum_h softmax(q keys_h^T) (states Wv_h Wc_h) + bc_eff.

Sharding: batch-parallel, one batch element per NeuronCore (8 cores).

Design notes:
  - fp16 matmul inputs: same 1 cycle/row PE rate as f32r but enables FWL
    (fast weight load) so LDWEIGHTS hides under matmuls, and 2x DVE/GpSimd
    elementwise rates + half the DMA bytes. fp16 (10-bit mantissa) over
    bf16 for accuracy margin; all tensors here are O(1) so no range risk.
  - Wc folded into Wv on host: vc_h = states @ (Wv_h Wc_h), so the per-head
    normalized context IS the head's output contribution; the final [H*E,E]
    projection matmuls, Wc DMA, and their PSUM traffic disappear.
  - bk dropped on device: softmax over k is invariant to the per-q additive
    shift (bk . q), so the keys bias cancels exactly.
  - bv folded into bc on host: sum_k attn = 1 implies ctx = ctx0 + bv, so
    out = ctx0 @ Wc + (bc + bv.flatten() @ Wc). Removes all vals-bias work.
  - scores/keys/vals/transposes share a 4-bank PSUM pool (phases disjoint);
    denominators 2 banks; ctx accumulators 2 banks. 8 banks total.
  - per head, the kt loop emits scores+exp for kt and ctx matmuls for kt-1
    (software pipelining) so PE never waits on ACT's exp.
  - head h's normalize (recip + mul + accumulate into final) is deferred
    until after head h+1's keys so DVE latency hides under PE matmuls.
"""

import sys

for _p in ("/opt/trn_rl_repo",):
    if _p not in sys.path:
        sys.path.insert(0, _p)

import numpy as np

import concourse.bass as bass
import concourse.mybir as mybir
import concourse.tile as tile
from concourse import bacc, bass_utils
from concourse.masks import make_identity

B, SQ, SK = 8, 1024, 1024
D, E, H = 1024, 128, 32
P = 128          # partition width
DCH = D // P     # 8 d-chunks
KT = SK // P     # 8 k-tiles
G = 4            # heads per vals-group
NG = H // G      # 8 groups
NHALF = 512      # matmul moving-dim chunk (one PSUM bank of fp32)
SCALE = 1.0 / float(np.sqrt(E))

F32 = mybir.dt.float32
BF16 = mybir.dt.bfloat16

N_CORES = 8

_COMPILED = {}
import ml_dtypes
_ONES_SQ = np.ones((P, P), ml_dtypes.bfloat16)


def build_nc(mm_dtype="bf16", repeat=1):
    """Build the single-core Bass program (SPMD across 8 cores).

    repeat > 1 re-emits the whole computation that many times (identical
    work each pass) for launch-overhead-amortized timing; the final DRAM
    output is written by every pass (all identical).
    """
    MT = BF16

    nc = bacc.Bacc("TRN2", target_bir_lowering=False, debug=False)

    statesT = nc.dram_tensor("statesT", [D, SK], MT, kind="ExternalInput").ap()
    queryT = nc.dram_tensor("queryT", [E, SQ], MT, kind="ExternalInput").ap()
    WkT = nc.dram_tensor("WkT", [NG, D, G * E], MT, kind="ExternalInput").ap()
    WvcT = nc.dram_tensor("WvcT", [NG, D, G * E], MT, kind="ExternalInput").ap()
    bcT = nc.dram_tensor("bcT", [E, 1], F32, kind="ExternalInput").ap()
    onesSQ = nc.dram_tensor("onesSQ", [P, P], MT, kind="ExternalInput").ap()
    # out is [E, SQ] (head-features on partitions); host transposes for free
    out = nc.dram_tensor("out", [E, SQ], F32, kind="ExternalOutput").ap()

    from contextlib import ExitStack

    with tile.TileContext(nc) as tc, ExitStack() as es:
        constp = es.enter_context(tc.tile_pool(name="const", bufs=1))
        statesp = es.enter_context(tc.tile_pool(name="states", bufs=DCH))
        queryp = es.enter_context(tc.tile_pool(name="query", bufs=1))
        wkp = es.enter_context(tc.tile_pool(name="wk", bufs=10))
        wvp = es.enter_context(tc.tile_pool(name="wv", bufs=10))
        keysp = es.enter_context(tc.tile_pool(name="keys", bufs=2))
        expp = es.enter_context(tc.tile_pool(name="exps", bufs=8))
        valsp = es.enter_context(tc.tile_pool(name="vals", bufs=12))
        recipp = es.enter_context(tc.tile_pool(name="recip", bufs=2))
        tmpp = es.enter_context(tc.tile_pool(name="tmpn", bufs=2))
        finalp = es.enter_context(tc.tile_pool(name="final", bufs=1))
        dsump = es.enter_context(tc.tile_pool(name="dsum", bufs=12))
        ps_main = es.enter_context(tc.tile_pool(name="ps_main", bufs=4, space="PSUM"))
        ps_ctx = es.enter_context(tc.tile_pool(name="ps_ctx", bufs=4, space="PSUM"))

        # ---- constants ----
        ones_sq = constp.tile([P, P], MT)
        nc.sync.dma_start(ones_sq[:], onesSQ[:])
        bc_t = constp.tile([E, 1], F32)
        nc.sync.dma_start(bc_t[:], bcT[:])

        # ---- resident activations ----
        st = []
        for d in range(DCH):
            st_t = statesp.tile([P, SK], MT, name="st_t")
            nc.sync.dma_start(st_t[:], statesT[d * P : (d + 1) * P, :])
            st.append(st_t)
        q_t = queryp.tile([E, SQ], MT)
        nc.sync.dma_start(q_t[:], queryT[:])

        final_t = finalp.tile([E, SQ], F32)

        def emit_norm_qh(h, pc, ssums, qh):
            """Denominator matmul + normalize one q-half of head h into final_t.

            Deferred into the NEXT head's kt loop (qh0 at kt1, qh1 at kt2) so
            the PE queue never stalls on the exp-reduction tree, and so the
            transient pd bank comes from the ps_main rotation.
            """
            pd = ps_main.tile([P, NHALF], F32, tag="m", name="pd")
            nc.tensor.matmul(
                pd[:], (ones_sq[:]), (ssums[qh][:]), start=True, stop=True
            )
            rec = recipp.tile([P, NHALF], F32, name="rec")
            nc.vector.reciprocal_approx_fast(out=rec[:], in_=pd[:])
            tmp = tmpp.tile([P, NHALF], F32, name="tmp")
            nc.vector.tensor_mul(tmp[:], pc[qh][:], rec[:])
            if h == 0:
                nc.vector.tensor_scalar(
                    final_t[:, qh * NHALF : (qh + 1) * NHALF],
                    tmp[:],
                    bc_t[:],
                    None,
                    op0=mybir.AluOpType.add,
                )
            else:
                nc.vector.tensor_add(
                    final_t[:, qh * NHALF : (qh + 1) * NHALF],
                    final_t[:, qh * NHALF : (qh + 1) * NHALF],
                    tmp[:],
                )

        pending_norm = None

        for rep in range(repeat):
            def emit_wdma(gw):
                wvt, wkt = [], []
                for d in range(DCH):
                    wv_t = wvp.tile([P, G * E], MT, name="wv_t")
                    nc.sync.dma_start(wv_t[:], WvcT[gw, d * P : (d + 1) * P, :])
                    wvt.append(wv_t)
                for d in range(DCH):
                    wk_t = wkp.tile([P, G * E], MT, name="wk_t")
                    nc.sync.dma_start(wk_t[:], WkT[gw, d * P : (d + 1) * P, :])
                    wkt.append(wk_t)
                return wvt, wkt

            def emit_vals_tile(kt, wvt):
                pv = ps_main.tile([P, G * E], F32, tag="m", name="pv")
                for d in range(DCH):
                    nc.tensor.matmul(
                        pv[:],
                        (st[d][:, kt * P : (kt + 1) * P]),
                        (wvt[d][:]),
                        start=(d == 0),
                        stop=(d == DCH - 1),
                    )
                v_sb = valsp.tile([P, G * E], MT, name="v_sb")
                nc.vector.tensor_copy(v_sb[:], pv[:])
                return v_sb

            next_w = emit_wdma(0)
            next_vals = []

            for g in range(NG):
                wv_tiles, wk_tiles = next_w
                # vals tiles not already built during the previous group's
                # last head:
                vals_tiles = next_vals
                for kt in range(len(vals_tiles), KT):
                    vals_tiles.append(emit_vals_tile(kt, wv_tiles))
                next_vals = []

                def emit_keys_block(hg_k):
                    """Full 16-matmul keys emission for head (g, hg_k)."""
                    ksb = keysp.tile([E, SK], MT, name="keys_sb")
                    for half in range(2):
                        pk = ps_main.tile(
                            [P, NHALF], F32, tag="m", name="pk"
                        )
                        for d in range(DCH):
                            nc.tensor.matmul(
                                pk[:],
                                (wk_tiles[d][:, hg_k * E : (hg_k + 1) * E]),
                                (st[d][:, half * NHALF : (half + 1) * NHALF]),
                                start=(d == 0),
                                stop=(d == DCH - 1),
                            )
                        nc.vector.tensor_copy(
                            ksb[:, half * NHALF : (half + 1) * NHALF], pk[:]
                        )
                    return ksb

                # first head's keys as one block (nothing to hide them under)
                next_keys_sb = emit_keys_block(0)

                for hg in range(G):
                    h = g * G + hg
                    keys_sb = next_keys_sb

                    # next head's keys are interleaved into this head's kt
                    # loop (2 matmuls per kt) to keep PE saturated while the
                    # softmax phase is ACT-paced. kts 0-3 build half 0,
                    # kts 4-7 half 1.
                    ik_hg = hg + 1 if hg + 1 < G else None
                    if ik_hg is not None:
                        next_keys_sb = keysp.tile([E, SK], MT, name="keys_sb")
                        ik_pk = None
                    iv_w = None
                    if ik_hg is None and g + 1 < NG:
                        next_w = emit_wdma(g + 1)
                        iv_w = next_w[0]
                        iv_pv = None

                    # ---- scores -> exp -> denom/ctx accumulate, per (kt, qh) ----
                    pc = [
                        ps_ctx.tile(
                            [P, NHALF], F32, tag="ctx", name="pc"
                        )
                        for i in range(2)
                    ]
                    # Software-pipelined: emit kt's scores+exp, then kt-1's
                    # ctx matmuls (so PE never waits on the just-issued exp).
                    # Denominator: gpsimd pair-adds of exp tiles as pairs
                    # complete (Pool engine is otherwise idle), DVE finishes
                    # the reduction tree, then ONE ones-matmul per half.
                    pending = None  # ([ex_qh0, ex_qh1], kt)
                    last_ex = [None, None]
                    partials = [[], []]
                    for kt in range(KT):
                        if kt in (1, 2) and pending_norm is not None:
                            # previous head's denominator matmul + normalize,
                            # deferred here so PE never stalls on its tree.
                            emit_norm_qh(*pending_norm, kt - 1)
                            if kt == 2:
                                pending_norm = None
                        exs = []
                        for qh in range(2):
                            ps = ps_main.tile(
                                [P, NHALF], F32, tag="m",
                                name="ps",
                            )
                            nc.tensor.matmul(
                                ps[:],
                                (keys_sb[:, kt * P : (kt + 1) * P]),
                                (q_t[:, qh * NHALF : (qh + 1) * NHALF]),
                                start=True,
                                stop=True,
                            )
                            ex = expp.tile(
                                [P, NHALF], MT, name="ex"
                            )
                            nc.scalar.activation(
                                ex[:], ps[:], mybir.ActivationFunctionType.Exp,
                                scale=SCALE,
                            )
                            exs.append(ex)
                            if kt % 2 == 1:
                                gsum = dsump.tile([P, NHALF], MT, name="gsum")
                                nc.vector.tensor_add(
                                    gsum[:], last_ex[qh][:], ex[:]
                                )
                                partials[qh].append(gsum)
                            else:
                                last_ex[qh] = ex
                        if iv_w is not None:
                            # 2 vals matmuls of group g+1 per kt: tile kt//4,
                            # d-chunks 2*(kt%4) and 2*(kt%4)+1
                            vkt = kt // 4
                            loc = kt % 4
                            if loc == 0:
                                iv_pv = ps_main.tile(
                                    [P, G * E], F32, tag="m", name="pv"
                                )
                            for d in (2 * loc, 2 * loc + 1):
                                nc.tensor.matmul(
                                    iv_pv[:],
                                    (st[d][:, vkt * P : (vkt + 1) * P]),
                                    (iv_w[d][:]),
                                    start=(d == 0),
                                    stop=(d == DCH - 1),
                                )
                            if loc == 3:
                                v_sb = valsp.tile([P, G * E], MT, name="v_sb")
                                nc.vector.tensor_copy(v_sb[:], iv_pv[:])
                                next_vals.append(v_sb)
                        if ik_hg is not None:
                            half = kt // 4
                            loc = kt % 4
                            if loc == 0:
                                ik_pk = ps_main.tile(
                                    [P, NHALF], F32, tag="m", name="pk"
                                )
                            for d in (2 * loc, 2 * loc + 1):
                                nc.tensor.matmul(
                                    ik_pk[:],
                                    (wk_tiles[d][:, ik_hg * E : (ik_hg + 1) * E]),
                                    (st[d][:, half * NHALF : (half + 1) * NHALF]),
                                    start=(d == 0),
                                    stop=(d == DCH - 1),
                                )
                            if loc == 3:
                                nc.vector.tensor_copy(
                                    next_keys_sb[
                                        :, half * NHALF : (half + 1) * NHALF
                                    ],
                                    ik_pk[:],
                                )
                        if pending is not None:
                            pexs, pkt = pending
                            for qh in range(2):
                                nc.tensor.matmul(
                                    pc[qh][:],
                                    (vals_tiles[pkt][:, hg * E : (hg + 1) * E]),
                                    (pexs[qh][:]),
                                    start=(pkt == 0),
                                    stop=False,
                                )
                        pending = (exs, kt)
                    pexs, pkt = pending
                    for qh in range(2):
                        nc.tensor.matmul(
                            pc[qh][:],
                            (vals_tiles[pkt][:, hg * E : (hg + 1) * E]),
                            (pexs[qh][:]),
                            start=False,
                            stop=True,
                        )
                    ssums = []
                    for qh in range(2):
                        p0, p1, p2, p3 = partials[qh]
                        d1 = dsump.tile([P, NHALF], MT, name="gsum")
                        nc.vector.tensor_add(d1[:], p0[:], p1[:])
                        d2 = dsump.tile([P, NHALF], MT, name="gsum")
                        nc.vector.tensor_add(d2[:], p2[:], p3[:])
                        ssum = dsump.tile([P, NHALF], MT, name="gsum")
                        nc.vector.tensor_add(ssum[:], d1[:], d2[:])
                        ssums.append(ssum)

                    # defer denom matmuls + normalize into next head's kt loop
                    pending_norm = (h, pc, ssums)

            emit_norm_qh(*pending_norm, 0)
            emit_norm_qh(*pending_norm, 1)
            pending_norm = None

            # ---- write out [E, SQ]; host transposes to [SQ, E] ----
            nc.sync.dma_start(out[:], final_t[:])

    nc.compile()
    return nc


def _prep_inputs(query, states, Wk, bk, Wv, bv, Wc, bc):
    """Host-side sharding: per-core input maps (core c == batch element c).

    bk is dropped (softmax shift invariance); bv is folded into bc; Wc is
    folded into Wv (Wvc_h = Wv_h @ Wc_h):
    out = sum_h attn_h @ (states @ Wvc_h) + (bc + bv.flatten() @ Wc).
    """
    query = np.asarray(query, np.float32)
    states = np.asarray(states, np.float32)
    Wk = np.asarray(Wk, np.float64)
    Wv = np.asarray(Wv, np.float64)
    Wc = np.asarray(Wc, np.float64)
    bv = np.asarray(bv, np.float64)
    bc = np.asarray(bc, np.float64)

    # Wvc[h] = Wv[h] @ Wc[h*E:(h+1)*E, :]  -> [H, D, E]
    Wvc = np.einsum("hde,hef->hdf", Wv, Wc.reshape(H, E, E))

    WkT = np.ascontiguousarray(
        Wk.transpose(1, 0, 2).reshape(D, NG, G * E).transpose(1, 0, 2)
    ).astype(ml_dtypes.bfloat16)
    WvcT = np.ascontiguousarray(
        Wvc.transpose(1, 0, 2).reshape(D, NG, G * E).transpose(1, 0, 2)
    ).astype(ml_dtypes.bfloat16)
    bc_eff = (bc + bv.reshape(H * E) @ Wc).astype(np.float32)
    bcT = np.ascontiguousarray(bc_eff.reshape(E, 1))

    in_maps = []
    for c in range(N_CORES):
        in_maps.append(
            {
                "statesT": np.ascontiguousarray(states[c].T).astype(ml_dtypes.bfloat16),
                "queryT": np.ascontiguousarray(query[c].T).astype(ml_dtypes.bfloat16),
                "WkT": WkT,
                "WvcT": WvcT,
                "bcT": bcT,
                "onesSQ": _ONES_SQ,
            }
        )
    return in_maps


def get_nc(mm_dtype="bf16", repeat=1):
    key = (mm_dtype, repeat)
    nc = _COMPILED.get(key)
    if nc is None:
        nc = build_nc(mm_dtype, repeat=repeat)
        _COMPILED[key] = nc
    return nc


def kernel(query, states, Wk, bk, Wv, bv, Wc, bc):
    nc = get_nc()
    in_maps = _prep_inputs(query, states, Wk, bk, Wv, bv, Wc, bc)
    res = bass_utils.run_bass_kernel_spmd(nc, in_maps, list(range(N_CORES)))
    return np.stack(
        [np.ascontiguousarray(res.results[c]["out"].T) for c in range(N_CORES)],
        axis=0,
    )
